# revision 22
# baseline (speedup 1.0000x reference)
"""Bass/Trainium2 kernel for nn_ExtractModel (soft banded edit-distance vocab matcher).

Sharding: vocab axis V=1000 split 8 x 125 across NeuronCores (partition dim = vocab).

Per core the device program computes, fully on-chip:
  PE   : band-only cosine matmuls in fp16 (psum = -0.5*dot), reading the padded
         word tensor with overlapping-window access patterns (no materialized
         [L, MSL] window tensor), then transposes of the reduced planes.
  ACT  : psum->sbuf copies producing diff = 0.5 - 0.5*dot in fp16 (bias fold).
  DVE  : banded soft edit-distance DP over the 36 band cells in fp16 (2x mode),
         the vocab_length-gather (per-partition mask scalars) producing
         per-(e,m) min candidates, and the final min-over-vocab reductions of
         the PE-transposed planes.
  The lens "key" planes (value + tau*vl[v]) cost one DVE add per plane since
  the packing term is j-independent; the host decodes vocab_length at the
  argmin (lens) from (keymin - valmin)/tau.
Device output per core is only [96, 40] fp32 (valmin + keymin per (e, m-half)).
Host does the tiny cross-core min + scoring/argmax (negligible).

The reference's second DP table (not_viable init, all-BIG) provably yields
values >= 99.9 > MATCH_THRESH everywhere, so non-viable positions always score
exactly +/-0.0 and never match; constant BIG gives identical final outputs.

Raw Bass (no TileContext): this toolchain's walrus rejects instructions
carrying more than one attached semaphore wait, so all cross-engine syncs are
standalone wait_ge instructions with a hand-rolled semaphore protocol.
Shapes hardcoded per the problem spec.
"""

import numpy as np

import concourse.bass as bass
import concourse.mybir as mybir

MSL = 10
MTL = 10
BIG = 99.9
MATCH_THRESH = 0.05
BS, L, D, V = 4, 48, 256, 1000
NCORES = 8
VC = V // NCORES          # 125 vocab words per core
M = BS * L                # 192 (b,s) positions
MH = M // 2               # 96, transpose chunk
KC = D // 128             # 2 contraction chunks
LP = L + MSL - 1          # 57 padded word positions
NB = 5                    # psum banks rotated by matmul groups
BIGM = 30000.0            # mask "off" value (fits fp16)
TAU = 2.0 ** -9           # lens packing step (fp16-safe)
F32 = mybir.dt.float32
F16 = mybir.dt.float16
Alu = mybir.AluOpType

# band cells of the edit-distance DP, row-major (dependency order)
BAND = [(i, j) for i in range(1, MSL + 1)
        for j in range(max(i - 2, 1), min(i + 2, MTL + 1))]
# per-column / per-row index lists (contiguous by construction)
ILIST = {j: sorted(i for (i, j2) in BAND if j2 == j) for j in range(1, MTL + 1)}
JLIST = {i: sorted(j for (i2, j) in BAND if i2 == i) for i in range(1, MSL + 1)}
# storage order: column-major so per-j slices are contiguous
BANDC = [(i, j) for j in range(1, MTL + 1) for i in ILIST[j]]
NCOL = {c: n for n, c in enumerate(BANDC)}

# matmul groups: per j, chunks of <=2 consecutive i's; emission order sorted
# by first DP use (i0, j)
_groups = []
for j in range(1, MTL + 1):
    il = ILIST[j]
    k = 0
    while k < len(il):
        w = 2 if k + 1 < len(il) and il[k + 1] == il[k] + 1 else 1
        _groups.append((il[k], j, w))
        k += w
GROUPS = sorted(_groups, key=lambda g: (g[0], g[1]))
NG = len(GROUPS)
GIDX = {}  # (i, j) -> emission index of covering group
for _gi, (_i0, _j, _w) in enumerate(GROUPS):
    for _i in range(_i0, _i0 + _w):
        GIDX[(_i, _j)] = _gi

_cache = {}


def _windows(ap, dim, stride, size):
    """Return a copy of `ap` with dims[dim] replaced by [stride, size]
    (raw access-pattern surgery, e.g. for overlapping windows)."""
    c = ap.copy()
    a = c.ap
    a[dim] = [stride, size]
    c.ap = a
    return c


def _build_program():
    nc = bass.Bass()
    wrpadT = nc.dram_tensor("wrpadT", [128, KC, BS, LP], F16, kind="ExternalInput")
    vocT = nc.dram_tensor("vocT", [128, KC, MTL, VC], F16, kind="ExternalInput")
    maskT = nc.dram_tensor("maskT", [VC, MTL + 1], F32, kind="ExternalInput")
    idT16 = nc.dram_tensor("idT16", [128, 128], F16, kind="ExternalInput")
    outT = nc.dram_tensor("outT", [MH, 40], F32, kind="ExternalOutput")

    import contextlib
    with contextlib.ExitStack() as ctx:
        ent = ctx.enter_context
        wr_t = ent(nc.sbuf_tensor("wr_t", [128, KC, BS, LP], F16))
        voc_t = ent(nc.sbuf_tensor("voc_t", [128, KC, MTL, VC], F16))
        mask_t = ent(nc.sbuf_tensor("mask_t", [VC, MTL + 1], F32))
        id16 = ent(nc.sbuf_tensor("id16", [128, 128], F16))
        diff = ent(nc.sbuf_tensor("diff", [VC, 36, M], F16))
        fall = ent(nc.sbuf_tensor("fall", [VC, 36, M], F16))
        vsel = ent(nc.sbuf_tensor("vsel", [VC, MSL, M], F16))
        ksel = ent(nc.sbuf_tensor("ksel", [VC, MSL, M], F16))
        tmpA = ent(nc.sbuf_tensor("tmpA", [VC, M], F16))
        tmpB = ent(nc.sbuf_tensor("tmpB", [VC, M], F16))
        outb = ent(nc.sbuf_tensor("outb", [MH, 40], F32))
        pb = [ent(nc.psum_tensor(f"pb{b}", [128, 512], F32)) for b in range(NB)]
        pbv = [ent(nc.psum_tensor(f"pbv{b}", [128, 8, 128], F16))
               for b in range(3)]
        s_in = ent(nc.semaphore("s_in"))
        s_in2 = ent(nc.semaphore("s_in2"))
        s_in3 = ent(nc.semaphore("s_in3"))
        s_pe = ent(nc.semaphore("s_pe"))
        s_actd = ent(nc.semaphore("s_actd"))
        s_vm = ent(nc.semaphore("s_vm"))
        s_tp = ent(nc.semaphore("s_tp"))
        s_k2 = ent(nc.semaphore("s_k2"))
        s_tpk = ent(nc.semaphore("s_tpk"))
        s_vfree = ent(nc.semaphore("s_vfree"))
        s_dve = ent(nc.semaphore("s_dve"))
        s_out = ent(nc.semaphore("s_out"))

        # transpose tile banks: vmin tiles 0-7 -> pbv0, 8-15 -> pbv1,
        # 16-19 -> pbv2 slots 0-3; keysel tiles 0-7 -> pbv0 (after vred0),
        # 8-15 -> pbv1 (after vred1), 16-19 -> pbv2 slots 4-7.
        def vtile(t):
            if t < 8:
                return pbv[0], t
            if t < 16:
                return pbv[1], t - 8
            return pbv[2], t - 16

        def ktile(t):
            if t < 8:
                return pbv[0], t
            if t < 16:
                return pbv[1], t - 8
            return pbv[2], t - 12      # slots 4-7

        with nc.Block() as block:

            @block.sync
            def _(sync):
                # s_in: wrpad + first vocab chunk (PE can start);
                # s_in2: rest of vocab; s_in3: masks + identity (DVE/PE late)
                sync.dma_start(wr_t[:], wrpadT[:]).then_inc(s_in, 16)
                sync.dma_start(voc_t[:, :, 0:3, :],
                               vocT[:, :, 0:3, :]).then_inc(s_in, 16)
                sync.dma_start(voc_t[:, :, 3:MTL, :],
                               vocT[:, :, 3:MTL, :]).then_inc(s_in2, 16)
                sync.dma_start(mask_t[:], maskT[:]).then_inc(s_in3, 16)
                sync.dma_start(id16[:], idT16[:]).then_inc(s_in3, 16)
                sync.wait_ge(s_vfree, 2)
                sync.wait_ge(s_dve, 4)
                sync.dma_start(outT[:], outb[:]).then_inc(s_out, 16)
                sync.wait_ge(s_out, 16)

            @block.tensor
            def _(tensor):
                tensor.wait_ge(s_in, 32)
                waited_full = False
                for gi, (i0, j, w) in enumerate(GROUPS):
                    if j > 3 and not waited_full:
                        tensor.wait_ge(s_in2, 16)
                        waited_full = True
                    if gi >= NB:
                        tensor.wait_ge(s_actd, gi - NB + 1)
                    pa = pb[gi % NB][0:VC, 0:w * M]
                    for kc in range(KC):
                        # moving operand: padded words with overlapping
                        # i-windows, dims (i:w stride 1, b:4, s:48)
                        mv = _windows(
                            wr_t[:, kc, :, i0 - 1:i0 - 1 + L].unsqueeze(1),
                            1, 1, w)
                        mm = tensor.matmul(
                            pa, voc_t[:, kc, j - 1, :], mv,
                            start=(kc == 0), stop=(kc == KC - 1),
                        )
                    mm.then_inc(s_pe, 1)

                # plane transposes (fp16), interleaved so keysel tiles
                # reuse pbv banks right after the vmin reductions free them:
                #   vm0..vm4, k0..k3 (pbv0, after vred0), vm5..vm8,
                #   k4..k7 (pbv1, after vred1), vm9, k8, k9 (pbv2 slots 4-7)
                def emit_vm_t(e):
                    tensor.wait_ge(s_vm, e + 1)
                    for mc in range(2):
                        bank, slot = vtile(2 * e + mc)
                        tensor.matmul(
                            bank[0:MH, slot, 0:VC],
                            vsel[:, e, mc * MH:(mc + 1) * MH],
                            id16[0:VC, 0:VC],
                            is_transpose=True,
                        ).then_inc(s_tp, 1)

                def emit_k_t(e):
                    tensor.wait_ge(s_k2, e + 1)
                    for mc in range(2):
                        bank, slot = ktile(2 * e + mc)
                        tensor.matmul(
                            bank[0:MH, slot, 0:VC],
                            ksel[:, e, mc * MH:(mc + 1) * MH],
                            id16[0:VC, 0:VC],
                            is_transpose=True,
                        ).then_inc(s_tpk, 1)

                tensor.wait_ge(s_in3, 32)
                for e in range(5):
                    emit_vm_t(e)
                tensor.wait_ge(s_vfree, 1)             # pbv0 consumed
                for e in range(4):
                    emit_k_t(e)
                for e in range(5, 9):
                    emit_vm_t(e)
                tensor.wait_ge(s_vfree, 2)             # pbv1 consumed
                for e in range(4, 8):
                    emit_k_t(e)
                emit_vm_t(9)
                emit_k_t(8)
                emit_k_t(9)

            @block.scalar
            def _(scalar):
                for gi, (i0, j, w) in enumerate(GROUPS):
                    scalar.wait_ge(s_pe, gi + 1)
                    n0 = NCOL[(i0, j)]
                    # diffm1 = -0.5*dot - 0.5 = diff - 1 (fplus DP transform)
                    scalar.activation(
                        diff[:, n0:n0 + w, :], pb[gi % NB][0:VC, 0:w * M],
                        mybir.ActivationFunctionType.Copy, bias=-0.5, scale=1.0,
                    ).then_inc(s_actd, 1)


            @block.vector
            def _(vector):
                # fplus transform: cells hold f+1 so the ins/del pred mins are
                # plain tensor_tensor(min) (2x mode) instead of
                # scalar_tensor_tensor (1x); diff tiles hold diff-1; the final
                # +1 is a cheap tensor_scalar; vmin masks are shifted by -1
                # host-side so outputs are unchanged.
                fmap = {}

                def pred(i, j):
                    """fplus predecessor: f(i,j)+1, or BIG if out of band."""
                    if (i, j) in fmap:
                        return fmap[(i, j)]
                    if i == 0:
                        return float(j) + 1.0
                    if j == 0:
                        return float(i) + 1.0
                    return BIG  # out of band

                state = {"waited": 0}

                def emit_cell(i, j):
                    # fplus(i,j) = min(P_sub + diffm1, P_ins, P_del) + 1
                    dijm1 = diff[:, NCOL[(i, j)], :]
                    need = GIDX[(i, j)] + 1
                    if need > state["waited"]:
                        vector.wait_ge(s_actd, need)
                        state["waited"] = need

                    sub_p = pred(i - 1, j - 1)
                    consts = [p for p in (pred(i - 1, j), pred(i, j - 1))
                              if isinstance(p, float) and p < BIG]
                    tens = [p for p in (pred(i - 1, j), pred(i, j - 1))
                            if not isinstance(p, float)]
                    mconst = min(consts) if consts else None

                    fcell = fall[:, NCOL[(i, j)], :]
                    fmap[(i, j)] = fcell
                    if isinstance(sub_p, float) and not tens:
                        # single fused op, +1 folded into both scalars
                        return vector.tensor_scalar(
                            fcell, dijm1, sub_p + 1.0,
                            (mconst + 1.0) if mconst is not None else BIG,
                            Alu.add, Alu.min)
                    tmps = [tmpA, tmpB]
                    k = 0
                    if isinstance(sub_p, float):
                        if mconst is not None:
                            acc = vector.tensor_scalar(
                                tmps[0][:], dijm1, sub_p, mconst,
                                Alu.add, Alu.min)
                        else:
                            acc = vector.tensor_scalar_add(
                                tmps[0][:], dijm1, sub_p)
                    else:
                        acc = vector.tensor_add(tmps[0][:], sub_p, dijm1)
                    cur = tmps[0][:]
                    for t in tens:
                        k += 1
                        out = tmps[k % 2][:]
                        vector.tensor_tensor(out, cur, t, Alu.min)
                        cur = out
                    return vector.tensor_scalar_add(fcell, cur, 1.0)

                def emit_vmin(e):
                    i = e + 1
                    js = JLIST[i]
                    acc = None
                    last = None
                    for k, j in enumerate(js):
                        cell = fall[:, NCOL[(i, j)], :]
                        mcol = mask_t[:, j - 1:j]
                        out = (vsel[:, e, :] if k == len(js) - 1
                               else [tmpA, tmpB][k % 2][:])
                        if k == 0:
                            last = vector.tensor_scalar_add(out, cell, mcol)
                        else:
                            last = vector.scalar_tensor_tensor(
                                out, cell, mcol, acc, Alu.add, Alu.min)
                        acc = out
                    return last

                def emit_vred(batch):
                    # batch 0: tiles 0-7 (pbv0), 1: 8-15 (pbv1), 2: 16-19
                    # (pbv2 slots 0-3); batches 0/1 release their bank for
                    # keysel transposes via s_vfree
                    t0, t1 = (0, 8) if batch == 0 else \
                        ((8, 16) if batch == 1 else (16, 20))
                    bank = pbv[batch]
                    n = t1 - t0
                    vector.wait_ge(s_tp, t1)
                    vector.tensor_reduce(
                        outb[:, t0:t1], bank[0:MH, 0:n, 0:VC],
                        mybir.AxisListType.X, Alu.min,
                    ).then_inc(s_vfree if batch < 2 else s_dve, 1)

                def emit_kred(batch):
                    t0, t1 = (0, 8) if batch == 0 else \
                        ((8, 16) if batch == 1 else (16, 20))
                    bank = pbv[batch]
                    s0 = 0 if batch < 2 else 4
                    n = t1 - t0
                    vector.wait_ge(s_tpk, t1)
                    vector.tensor_reduce(
                        outb[:, 20 + t0:20 + t1],
                        bank[0:MH, s0:s0 + n, 0:VC],
                        mybir.AxisListType.X, Alu.min,
                    ).then_inc(s_dve, 1)

                for i in range(1, MSL + 1):
                    last = None
                    for j in JLIST[i]:
                        last = emit_cell(i, j)
                    if i == 1:
                        vector.wait_ge(s_in3, 32)  # mask_t landed
                    emit_vmin(i - 1).then_inc(s_vm, 1)
                    vector.tensor_scalar_add(
                        ksel[:, i - 1, :], vsel[:, i - 1, :],
                        mask_t[:, MTL:MTL + 1],
                    ).then_inc(s_k2, 1)
                    if i == 5:
                        emit_vred(0)
                    elif i == 7:
                        emit_kred(0)
                    elif i == 9:
                        emit_vred(1)
                emit_kred(1)
                emit_vred(2)
                emit_kred(2)

    return nc


def _get_runner():
    """Build the program + a cached jitted PJRT executor (built once)."""
    if "runner" in _cache:
        return _cache["runner"]
    import jax
    from jax.sharding import Mesh, PartitionSpec
    from jax.experimental.shard_map import shard_map
    from concourse.bass2jax import (_bass_exec_p, install_neuronx_cc_hook,
                                    partition_id_tensor)

    install_neuronx_cc_hook()
    nc = _build_program()

    part_name = (nc.partition_id_tensor.name
                 if nc.partition_id_tensor else None)
    in_names, out_names, out_avals, out_shapes = [], [], [], []
    for alloc in nc.m.functions[0].allocations:
        if not isinstance(alloc, mybir.MemoryLocationSet):
            continue
        name = alloc.memorylocations[0].name
        if alloc.kind == "ExternalInput":
            if name != part_name:
                in_names.append(name)
        elif alloc.kind == "ExternalOutput":
            shape = tuple(alloc.tensor_shape)
            dtype = mybir.dt.np(alloc.dtype)
            out_names.append(name)
            out_avals.append(jax.core.ShapedArray(shape, dtype))
            out_shapes.append((shape, dtype))
    n_params = len(in_names)
    n_outs = len(out_avals)
    all_names = in_names + out_names
    if part_name is not None:
        all_names = all_names + [part_name]
    donate = tuple(range(n_params, n_params + n_outs))

    def _body(*args):
        operands = list(args)
        if part_name is not None:
            operands.append(partition_id_tensor())
        outs = _bass_exec_p.bind(
            *operands,
            out_avals=tuple(out_avals),
            in_names=tuple(all_names),
            out_names=tuple(out_names),
            lowering_input_output_aliases=(),
            sim_require_finite=True,
            sim_require_nnan=True,
            nc=nc,
        )
        return tuple(outs)

    devices = jax.devices()[:NCORES]
    assert len(devices) >= NCORES, f"need {NCORES} devices"
    mesh = Mesh(np.asarray(devices), ("core",))
    in_specs = (PartitionSpec("core"),) * (n_params + n_outs)
    out_specs = (PartitionSpec("core"),) * n_outs
    sharded = jax.jit(
        shard_map(_body, mesh=mesh, in_specs=in_specs, out_specs=out_specs,
                  check_rep=False),
        donate_argnums=donate, keep_unused=True,
    )

    from jax.sharding import NamedSharding
    sh_core = NamedSharding(mesh, PartitionSpec("core"))

    def run(in_maps):
        """in_maps: per-core dicts. Device-put each input once per content
        (the axon host->device tunnel is ~30 MB/s; vocab alone is 5 MB and
        call-invariant in practice), then execute and read back in one go."""
        import zlib
        import jax as _jax
        concat_in = []
        for nm in in_names:
            arrs = [in_maps[c][nm] for c in range(NCORES)]
            key = (nm, tuple(zlib.adler32(np.ascontiguousarray(a).view(np.uint8).reshape(-1))
                             for a in arrs))
            hit = _cache.get(("dev", nm))
            if hit is not None and hit[0] == key:
                concat_in.append(hit[1])
                continue
            dev = _jax.device_put(np.concatenate(arrs, axis=0), sh_core)
            _cache[("dev", nm)] = (key, dev)
            concat_in.append(dev)
        concat_zeros = [
            np.zeros((NCORES * s[0], *s[1:]), dt) for (s, dt) in out_shapes
        ]
        out_arrs = sharded(*concat_in, *concat_zeros)
        host = [np.asarray(a) for a in out_arrs]
        return [
            {nm: host[i].reshape(NCORES, *out_shapes[i][0])[c]
             for i, nm in enumerate(out_names)}
            for c in range(NCORES)
        ]

    _cache["runner"] = (run, nc)
    return _cache["runner"]


def _host_prep(word_repr, vocab_repr, vocab_length):
    """Build per-core device inputs."""
    wr = np.asarray(word_repr, np.float32)
    vo = np.asarray(vocab_repr, np.float32)
    vl = np.asarray(vocab_length).astype(np.int64)

    # normalized words, padded with clamped copies: wrpad[b,p] = wrn[b,min(p,L-1)]
    nx = np.sqrt((wr * wr).sum(-1, dtype=np.float32)) + np.float32(1e-8)
    wrn = wr / nx[..., None]
    wrpad = np.concatenate(
        [wrn, np.repeat(wrn[:, L - 1:L, :], MSL - 1, axis=1)], axis=1)
    # layout [128(k), KC, BS, LP], d = kc*128 + k
    wrpadT = np.ascontiguousarray(
        wrpad.reshape(BS, LP, KC, 128).transpose(3, 2, 0, 1)).astype(np.float16)

    ny = np.sqrt((vo * vo).sum(-1, dtype=np.float32)) + np.float32(1e-8)
    von = vo * (np.float32(-0.5) / ny[..., None])          # fold the -0.5
    # per-core layout [128(k), KC, MTL, VC]
    vonT = np.ascontiguousarray(
        von.reshape(NCORES, VC, MTL, KC, 128).transpose(0, 4, 3, 2, 1)
    ).astype(np.float16)

    # masks (fplus-shifted): cmask[v, j-1] = -1 if vl[v]==j else BIGM ;
    # col 10 = TAU*vl
    jj = np.arange(1, MTL + 1)[None, :]
    cmask = np.where(vl[:, None] == jj, np.float32(-1), np.float32(BIGM))
    tau_col = (TAU * vl[:, None]).astype(np.float32)
    maskT = np.concatenate([cmask, tau_col], axis=1).astype(np.float32)

    if "ident" not in _cache:
        _cache["ident"] = np.eye(128, dtype=np.float16)
    id16 = _cache["ident"]

    return [{
        "wrpadT": wrpadT,
        "vocT": vonT[c],
        "maskT": maskT[c * VC:(c + 1) * VC],
        "idT16": id16,
    } for c in range(NCORES)]


def _finish(out, lengths):
    """out: [8, 96, 40] fp32 device results; returns the 4 model outputs."""
    vmin = np.empty((NCORES, MSL, M), np.float32)
    kmin = np.empty((NCORES, MSL, M), np.float32)
    for e in range(MSL):
        for mc in range(2):
            t = 2 * e + mc
            vmin[:, e, mc * MH:(mc + 1) * MH] = out[:, :, t]
            kmin[:, e, mc * MH:(mc + 1) * MH] = out[:, :, 20 + t]

    cstar = vmin.argmin(axis=0)                                # [10, 192]
    bv = np.take_along_axis(vmin, cstar[None], axis=0)[0]
    km = np.take_along_axis(kmin, cstar[None], axis=0)[0]
    lens = np.rint((km - bv) / np.float32(TAU)).astype(np.float32)
    lens = np.clip(lens, 0.0, float(MTL))

    # [e, m] -> [b, s, e]
    bv_bse = bv.reshape(MSL, BS, L).transpose(1, 2, 0)
    lens_bse = lens.reshape(MSL, BS, L).transpose(1, 2, 0)

    viable = (np.arange(L)[:, None] + np.arange(MSL)[None, :])[None] \
        < lengths[:, None, None]
    bvv = np.where(viable, bv_bse, np.float32(BIG))
    matched = bvv < np.float32(MATCH_THRESH)
    score = lens_bse * matched.astype(np.float32) * (np.float32(1.0) - bvv)

    sf = score.reshape(BS, -1)
    best_scores = sf.max(axis=-1).astype(np.float32)
    best_inds = sf.argmax(axis=-1).astype(np.int32)
    best_starts = best_inds // MSL
    best_ends = best_inds % MSL + best_starts
    matched_any = matched.reshape(BS, -1).any(axis=-1)
    return (best_scores, best_starts.astype(np.int32),
            best_ends.astype(np.int32), matched_any)


def kernel(word_repr, vocab_repr, lengths, vocab_length):
    lengths = np.asarray(lengths).astype(np.int64)
    in_maps = _host_prep(word_repr, vocab_repr, vocab_length)
    run, _nc = _get_runner()
    import time as _t
    t0 = _t.perf_counter()
    res = run(in_maps)
    _cache["last_device_s"] = _t.perf_counter() - t0
    _cache["last_in_maps"] = in_maps
    out = np.stack([res[c]["outT"] for c in range(NCORES)])
    return _finish(out, lengths)


# revision 24
# speedup vs baseline: 1.1016x; 1.1016x over previous
"""Bass/Trainium2 kernel for nn_ExtractModel (soft banded edit-distance vocab matcher).

Sharding: vocab axis V=1000 split 8 x 125 across NeuronCores (partition dim = vocab).

Per core the device program computes, fully on-chip:
  PE   : band-only cosine matmuls in fp16 (psum = -0.5*dot), reading the padded
         word tensor with overlapping-window access patterns (no materialized
         [L, MSL] window tensor), then transposes of the reduced planes.
  ACT  : psum->sbuf copies producing diff = 0.5 - 0.5*dot in fp16 (bias fold).
  DVE  : banded soft edit-distance DP over the 36 band cells in fp16 (2x mode),
         the vocab_length-gather (per-partition mask scalars) producing
         per-(e,m) min candidates, and the final min-over-vocab reductions of
         the PE-transposed planes.
  The lens "key" planes (value + tau*vl[v]) cost one DVE add per plane since
  the packing term is j-independent; the host decodes vocab_length at the
  argmin (lens) from (keymin - valmin)/tau.
Device output per core is only [96, 40] fp32 (valmin + keymin per (e, m-half)).
Host does the tiny cross-core min + scoring/argmax (negligible).

The reference's second DP table (not_viable init, all-BIG) provably yields
values >= 99.9 > MATCH_THRESH everywhere, so non-viable positions always score
exactly +/-0.0 and never match; constant BIG gives identical final outputs.

Raw Bass (no TileContext): this toolchain's walrus rejects instructions
carrying more than one attached semaphore wait, so all cross-engine syncs are
standalone wait_ge instructions with a hand-rolled semaphore protocol.
Shapes hardcoded per the problem spec.
"""

import numpy as np

import concourse.bass as bass
import concourse.mybir as mybir

MSL = 10
MTL = 10
BIG = 99.9
MATCH_THRESH = 0.05
BS, L, D, V = 4, 48, 256, 1000
NCORES = 8
VC = V // NCORES          # 125 vocab words per core
M = BS * L                # 192 (b,s) positions
MH = M // 2               # 96, transpose chunk
KC = D // 128             # 2 contraction chunks
LP = L + MSL - 1          # 57 padded word positions
NB = 5                    # psum banks rotated by matmul groups
BIGM = 30000.0            # mask "off" value (fits fp16)
TAU = 2.0 ** -9           # lens packing step (fp16-safe)
F32 = mybir.dt.float32
F16 = mybir.dt.float16
Alu = mybir.AluOpType

# band cells of the edit-distance DP, row-major (dependency order)
BAND = [(i, j) for i in range(1, MSL + 1)
        for j in range(max(i - 2, 1), min(i + 2, MTL + 1))]
# per-column / per-row index lists (contiguous by construction)
ILIST = {j: sorted(i for (i, j2) in BAND if j2 == j) for j in range(1, MTL + 1)}
JLIST = {i: sorted(j for (i2, j) in BAND if i2 == i) for i in range(1, MSL + 1)}
# storage order: column-major so per-j slices are contiguous
BANDC = [(i, j) for j in range(1, MTL + 1) for i in ILIST[j]]
NCOL = {c: n for n, c in enumerate(BANDC)}

# matmul groups: per j, chunks of <=2 consecutive i's; emission order sorted
# by first DP use (i0, j)
_groups = []
for j in range(1, MTL + 1):
    il = ILIST[j]
    k = 0
    while k < len(il):
        w = 2 if k + 1 < len(il) and il[k + 1] == il[k] + 1 else 1
        _groups.append((il[k], j, w))
        k += w
GROUPS = sorted(_groups, key=lambda g: (g[0], g[1]))
NG = len(GROUPS)
GIDX = {}  # (i, j) -> emission index of covering group
for _gi, (_i0, _j, _w) in enumerate(GROUPS):
    for _i in range(_i0, _i0 + _w):
        GIDX[(_i, _j)] = _gi

_cache = {}


def _windows(ap, dim, stride, size):
    """Return a copy of `ap` with dims[dim] replaced by [stride, size]
    (raw access-pattern surgery, e.g. for overlapping windows)."""
    c = ap.copy()
    a = c.ap
    a[dim] = [stride, size]
    c.ap = a
    return c


def _build_program():
    nc = bass.Bass()
    wrpadT = nc.dram_tensor("wrpadT", [128, KC, BS, LP], F16, kind="ExternalInput")
    vocT = nc.dram_tensor("vocT", [128, KC, MTL, VC], F16, kind="ExternalInput")
    maskT = nc.dram_tensor("maskT", [VC, MTL + 1], F32, kind="ExternalInput")
    idT16 = nc.dram_tensor("idT16", [128, 128], F16, kind="ExternalInput")
    outT = nc.dram_tensor("outT", [MH, 40], F32, kind="ExternalOutput")

    import contextlib
    with contextlib.ExitStack() as ctx:
        ent = ctx.enter_context
        wr_t = ent(nc.sbuf_tensor("wr_t", [128, KC, BS, LP], F16))
        voc_t = ent(nc.sbuf_tensor("voc_t", [128, KC, MTL, VC], F16))
        mask_t = ent(nc.sbuf_tensor("mask_t", [VC, MTL + 1], F32))
        id16 = ent(nc.sbuf_tensor("id16", [128, 128], F16))
        diff = ent(nc.sbuf_tensor("diff", [VC, 36, M], F16))
        fall = ent(nc.sbuf_tensor("fall", [VC, 36, M], F16))
        vsel = ent(nc.sbuf_tensor("vsel", [VC, MSL, M], F16))
        ksel = ent(nc.sbuf_tensor("ksel", [VC, MSL, M], F16))
        tmpA = ent(nc.sbuf_tensor("tmpA", [VC, M], F16))
        tmpB = ent(nc.sbuf_tensor("tmpB", [VC, M], F16))
        outb = ent(nc.sbuf_tensor("outb", [MH, 40], F32))
        pb = [ent(nc.psum_tensor(f"pb{b}", [128, 512], F32)) for b in range(NB)]
        pbv = [ent(nc.psum_tensor(f"pbv{b}", [128, 8, 128], F16))
               for b in range(3)]
        s_in = ent(nc.semaphore("s_in"))
        s_in2 = ent(nc.semaphore("s_in2"))
        s_in3 = ent(nc.semaphore("s_in3"))
        s_pe = ent(nc.semaphore("s_pe"))
        s_actd = ent(nc.semaphore("s_actd"))
        s_vm = ent(nc.semaphore("s_vm"))
        s_tp = ent(nc.semaphore("s_tp"))
        s_k2 = ent(nc.semaphore("s_k2"))
        s_tpk = ent(nc.semaphore("s_tpk"))
        s_vfree = ent(nc.semaphore("s_vfree"))
        s_dve = ent(nc.semaphore("s_dve"))
        s_out = ent(nc.semaphore("s_out"))

        # transpose tile banks: vmin tiles 0-7 -> pbv0, 8-15 -> pbv1,
        # 16-19 -> pbv2 slots 0-3; keysel tiles 0-7 -> pbv0 (after vred0),
        # 8-15 -> pbv1 (after vred1), 16-19 -> pbv2 slots 4-7.
        def vtile(t):
            if t < 8:
                return pbv[0], t
            if t < 16:
                return pbv[1], t - 8
            return pbv[2], t - 16

        def ktile(t):
            if t < 8:
                return pbv[0], t
            if t < 16:
                return pbv[1], t - 8
            return pbv[2], t - 12      # slots 4-7

        with nc.Block() as block:

            @block.sync
            def _(sync):
                # s_in: wrpad + first vocab chunk (PE can start);
                # s_in2: rest of vocab; s_in3: masks + identity (DVE/PE late)
                sync.dma_start(wr_t[:], wrpadT[:]).then_inc(s_in, 16)
                sync.dma_start(voc_t[:, :, 0:3, :],
                               vocT[:, :, 0:3, :]).then_inc(s_in, 16)
                sync.dma_start(voc_t[:, :, 3:MTL, :],
                               vocT[:, :, 3:MTL, :]).then_inc(s_in2, 16)
                sync.dma_start(mask_t[:], maskT[:]).then_inc(s_in3, 16)
                sync.dma_start(id16[:], idT16[:]).then_inc(s_in3, 16)
                sync.wait_ge(s_vfree, 2)
                sync.wait_ge(s_dve, 4)
                sync.dma_start(outT[:], outb[:]).then_inc(s_out, 16)
                sync.wait_ge(s_out, 16)

            @block.tensor
            def _(tensor):
                tensor.wait_ge(s_in, 32)
                waited_full = False
                for gi, (i0, j, w) in enumerate(GROUPS):
                    if j > 3 and not waited_full:
                        tensor.wait_ge(s_in2, 16)
                        waited_full = True
                    if gi >= NB:
                        tensor.wait_ge(s_actd, gi - NB + 1)
                    pa = pb[gi % NB][0:VC, 0:w * M]
                    for kc in range(KC):
                        # moving operand: padded words with overlapping
                        # i-windows, dims (i:w stride 1, b:4, s:48)
                        mv = _windows(
                            wr_t[:, kc, :, i0 - 1:i0 - 1 + L].unsqueeze(1),
                            1, 1, w)
                        mm = tensor.matmul(
                            pa, voc_t[:, kc, j - 1, :], mv,
                            start=(kc == 0), stop=(kc == KC - 1),
                        )
                    mm.then_inc(s_pe, 1)

                # plane transposes (fp16), interleaved so keysel tiles
                # reuse pbv banks right after the vmin reductions free them:
                #   vm0..vm4, k0..k3 (pbv0, after vred0), vm5..vm8,
                #   k4..k7 (pbv1, after vred1), vm9, k8, k9 (pbv2 slots 4-7)
                def emit_vm_t(e):
                    tensor.wait_ge(s_vm, e + 1)
                    for mc in range(2):
                        bank, slot = vtile(2 * e + mc)
                        tensor.matmul(
                            bank[0:MH, slot, 0:VC],
                            vsel[:, e, mc * MH:(mc + 1) * MH],
                            id16[0:VC, 0:VC],
                            is_transpose=True,
                        ).then_inc(s_tp, 1)

                def emit_k_t(e):
                    tensor.wait_ge(s_k2, e + 1)
                    for mc in range(2):
                        bank, slot = ktile(2 * e + mc)
                        tensor.matmul(
                            bank[0:MH, slot, 0:VC],
                            ksel[:, e, mc * MH:(mc + 1) * MH],
                            id16[0:VC, 0:VC],
                            is_transpose=True,
                        ).then_inc(s_tpk, 1)

                tensor.wait_ge(s_in3, 32)
                for e in range(5):
                    emit_vm_t(e)
                tensor.wait_ge(s_vfree, 1)             # pbv0 consumed
                for e in range(4):
                    emit_k_t(e)
                for e in range(5, 9):
                    emit_vm_t(e)
                tensor.wait_ge(s_vfree, 2)             # pbv1 consumed
                for e in range(4, 8):
                    emit_k_t(e)
                emit_vm_t(9)
                emit_k_t(8)
                emit_k_t(9)

            @block.scalar
            def _(scalar):
                for gi, (i0, j, w) in enumerate(GROUPS):
                    scalar.wait_ge(s_pe, gi + 1)
                    n0 = NCOL[(i0, j)]
                    # diffm1 = -0.5*dot - 0.5 = diff - 1 (fplus DP transform)
                    scalar.activation(
                        diff[:, n0:n0 + w, :], pb[gi % NB][0:VC, 0:w * M],
                        mybir.ActivationFunctionType.Copy, bias=-0.5, scale=1.0,
                    ).then_inc(s_actd, 1)



            @block.vector
            def _(vector):
                # fplus transform: cells hold f+1 so the ins/del pred mins are
                # plain tensor_tensor(min) (2x mode) instead of
                # scalar_tensor_tensor (1x); diff tiles hold diff-1; the final
                # +1 is a cheap tensor_scalar; vmin masks are shifted by -1
                # host-side so outputs are unchanged.
                fmap = {}

                def pred(i, j):
                    """fplus predecessor: f(i,j)+1, or BIG if out of band."""
                    if (i, j) in fmap:
                        return fmap[(i, j)]
                    if i == 0:
                        return float(j) + 1.0
                    if j == 0:
                        return float(i) + 1.0
                    return BIG  # out of band

                state = {"waited": 0}

                def emit_cell(i, j):
                    # fplus(i,j) = min(P_sub + diffm1, P_ins, P_del) + 1
                    dijm1 = diff[:, NCOL[(i, j)], :]
                    need = GIDX[(i, j)] + 1
                    if need > state["waited"]:
                        vector.wait_ge(s_actd, need)
                        state["waited"] = need

                    sub_p = pred(i - 1, j - 1)
                    consts = [p for p in (pred(i - 1, j), pred(i, j - 1))
                              if isinstance(p, float) and p < BIG]
                    tens = [p for p in (pred(i - 1, j), pred(i, j - 1))
                            if not isinstance(p, float)]
                    mconst = min(consts) if consts else None

                    fcell = fall[:, NCOL[(i, j)], :]
                    fmap[(i, j)] = fcell
                    if isinstance(sub_p, float) and not tens:
                        # single fused op, +1 folded into both scalars
                        return vector.tensor_scalar(
                            fcell, dijm1, sub_p + 1.0,
                            (mconst + 1.0) if mconst is not None else BIG,
                            Alu.add, Alu.min)
                    tmps = [tmpA, tmpB]
                    k = 0
                    if isinstance(sub_p, float):
                        if mconst is not None:
                            acc = vector.tensor_scalar(
                                tmps[0][:], dijm1, sub_p, mconst,
                                Alu.add, Alu.min)
                        else:
                            acc = vector.tensor_scalar_add(
                                tmps[0][:], dijm1, sub_p)
                    else:
                        acc = vector.tensor_add(tmps[0][:], sub_p, dijm1)
                    cur = tmps[0][:]
                    for t in tens:
                        k += 1
                        out = tmps[k % 2][:]
                        vector.tensor_tensor(out, cur, t, Alu.min)
                        cur = out
                    return vector.tensor_scalar_add(fcell, cur, 1.0)

                def emit_vmin(e):
                    i = e + 1
                    js = JLIST[i]
                    acc = None
                    last = None
                    for k, j in enumerate(js):
                        cell = fall[:, NCOL[(i, j)], :]
                        mcol = mask_t[:, j - 1:j]
                        out = (vsel[:, e, :] if k == len(js) - 1
                               else [tmpA, tmpB][k % 2][:])
                        if k == 0:
                            last = vector.tensor_scalar_add(out, cell, mcol)
                        else:
                            last = vector.scalar_tensor_tensor(
                                out, cell, mcol, acc, Alu.add, Alu.min)
                        acc = out
                    return last

                def emit_vred(batch):
                    # batch 0: tiles 0-7 (pbv0), 1: 8-15 (pbv1), 2: 16-19
                    # (pbv2 slots 0-3); batches 0/1 release their bank for
                    # keysel transposes via s_vfree
                    t0, t1 = (0, 8) if batch == 0 else \
                        ((8, 16) if batch == 1 else (16, 20))
                    bank = pbv[batch]
                    n = t1 - t0
                    vector.wait_ge(s_tp, t1)
                    vector.tensor_reduce(
                        outb[:, t0:t1], bank[0:MH, 0:n, 0:VC],
                        mybir.AxisListType.X, Alu.min,
                    ).then_inc(s_vfree if batch < 2 else s_dve, 1)

                def emit_kred(batch):
                    t0, t1 = (0, 8) if batch == 0 else \
                        ((8, 16) if batch == 1 else (16, 20))
                    bank = pbv[batch]
                    s0 = 0 if batch < 2 else 4
                    n = t1 - t0
                    vector.wait_ge(s_tpk, t1)
                    vector.tensor_reduce(
                        outb[:, 20 + t0:20 + t1],
                        bank[0:MH, s0:s0 + n, 0:VC],
                        mybir.AxisListType.X, Alu.min,
                    ).then_inc(s_dve, 1)

                for i in range(1, MSL + 1):
                    last = None
                    for j in JLIST[i]:
                        last = emit_cell(i, j)
                    if i == 1:
                        vector.wait_ge(s_in3, 32)  # mask_t landed
                    emit_vmin(i - 1).then_inc(s_vm, 1)
                    vector.tensor_scalar_add(
                        ksel[:, i - 1, :], vsel[:, i - 1, :],
                        mask_t[:, MTL:MTL + 1],
                    ).then_inc(s_k2, 1)
                    if i == 5:
                        emit_vred(0)
                    elif i == 7:
                        emit_kred(0)
                    elif i == 9:
                        emit_vred(1)
                emit_kred(1)
                emit_vred(2)
                emit_kred(2)

    return nc


def _get_runner():
    """Build the program + a cached jitted PJRT executor (built once)."""
    if "runner" in _cache:
        return _cache["runner"]
    import jax
    from jax.sharding import Mesh, PartitionSpec
    from jax.experimental.shard_map import shard_map
    from concourse.bass2jax import (_bass_exec_p, install_neuronx_cc_hook,
                                    partition_id_tensor)

    install_neuronx_cc_hook()
    nc = _build_program()

    part_name = (nc.partition_id_tensor.name
                 if nc.partition_id_tensor else None)
    in_names, out_names, out_avals, out_shapes = [], [], [], []
    for alloc in nc.m.functions[0].allocations:
        if not isinstance(alloc, mybir.MemoryLocationSet):
            continue
        name = alloc.memorylocations[0].name
        if alloc.kind == "ExternalInput":
            if name != part_name:
                in_names.append(name)
        elif alloc.kind == "ExternalOutput":
            shape = tuple(alloc.tensor_shape)
            dtype = mybir.dt.np(alloc.dtype)
            out_names.append(name)
            out_avals.append(jax.core.ShapedArray(shape, dtype))
            out_shapes.append((shape, dtype))
    n_params = len(in_names)
    n_outs = len(out_avals)
    all_names = in_names + out_names
    if part_name is not None:
        all_names = all_names + [part_name]
    donate = tuple(range(n_params, n_params + n_outs))

    def _body(*args):
        operands = list(args)
        if part_name is not None:
            operands.append(partition_id_tensor())
        outs = _bass_exec_p.bind(
            *operands,
            out_avals=tuple(out_avals),
            in_names=tuple(all_names),
            out_names=tuple(out_names),
            lowering_input_output_aliases=(),
            sim_require_finite=True,
            sim_require_nnan=True,
            nc=nc,
        )
        return tuple(outs)

    devices = jax.devices()[:NCORES]
    assert len(devices) >= NCORES, f"need {NCORES} devices"
    mesh = Mesh(np.asarray(devices), ("core",))
    in_specs = (PartitionSpec("core"),) * (n_params + n_outs)
    out_specs = (PartitionSpec("core"),) * n_outs
    sharded = jax.jit(
        shard_map(_body, mesh=mesh, in_specs=in_specs, out_specs=out_specs,
                  check_rep=False),
        donate_argnums=donate, keep_unused=True,
    )

    from jax.sharding import NamedSharding
    sh_core = NamedSharding(mesh, PartitionSpec("core"))

    def run(in_maps):
        """in_maps: per-core dicts. Device-put each input once per content
        (the axon host->device tunnel is ~30 MB/s; vocab alone is 5 MB and
        call-invariant in practice), then execute and read back in one go."""
        import zlib
        import jax as _jax
        concat_in = []
        for nm in in_names:
            arrs = [in_maps[c][nm] for c in range(NCORES)]
            key = (nm, tuple(zlib.adler32(np.ascontiguousarray(a).view(np.uint8).reshape(-1))
                             for a in arrs))
            hit = _cache.get(("dev", nm))
            if hit is not None and hit[0] == key:
                concat_in.append(hit[1])
                continue
            dev = _jax.device_put(np.concatenate(arrs, axis=0), sh_core)
            _cache[("dev", nm)] = (key, dev)
            concat_in.append(dev)
        concat_zeros = [
            np.zeros((NCORES * s[0], *s[1:]), dt) for (s, dt) in out_shapes
        ]
        out_arrs = sharded(*concat_in, *concat_zeros)
        host = [np.asarray(a) for a in out_arrs]
        return [
            {nm: host[i].reshape(NCORES, *out_shapes[i][0])[c]
             for i, nm in enumerate(out_names)}
            for c in range(NCORES)
        ]

    _cache["runner"] = (run, nc)
    return _cache["runner"]


def _host_prep(word_repr, vocab_repr, vocab_length):
    """Build per-core device inputs."""
    wr = np.asarray(word_repr, np.float32)
    vo = np.asarray(vocab_repr, np.float32)
    vl = np.asarray(vocab_length).astype(np.int64)

    # normalized words, padded with clamped copies: wrpad[b,p] = wrn[b,min(p,L-1)]
    nx = np.sqrt((wr * wr).sum(-1, dtype=np.float32)) + np.float32(1e-8)
    wrn = wr / nx[..., None]
    wrpad = np.concatenate(
        [wrn, np.repeat(wrn[:, L - 1:L, :], MSL - 1, axis=1)], axis=1)
    # layout [128(k), KC, BS, LP], d = kc*128 + k
    wrpadT = np.ascontiguousarray(
        wrpad.reshape(BS, LP, KC, 128).transpose(3, 2, 0, 1)).astype(np.float16)

    ny = np.sqrt((vo * vo).sum(-1, dtype=np.float32)) + np.float32(1e-8)
    von = vo * (np.float32(-0.5) / ny[..., None])          # fold the -0.5
    # per-core layout [128(k), KC, MTL, VC]
    vonT = np.ascontiguousarray(
        von.reshape(NCORES, VC, MTL, KC, 128).transpose(0, 4, 3, 2, 1)
    ).astype(np.float16)

    # masks (fplus-shifted): cmask[v, j-1] = -1 if vl[v]==j else BIGM ;
    # col 10 = TAU*vl
    jj = np.arange(1, MTL + 1)[None, :]
    cmask = np.where(vl[:, None] == jj, np.float32(-1), np.float32(BIGM))
    tau_col = (TAU * vl[:, None]).astype(np.float32)
    maskT = np.concatenate([cmask, tau_col], axis=1).astype(np.float32)

    if "ident" not in _cache:
        _cache["ident"] = np.eye(128, dtype=np.float16)
    id16 = _cache["ident"]

    return [{
        "wrpadT": wrpadT,
        "vocT": vonT[c],
        "maskT": maskT[c * VC:(c + 1) * VC],
        "idT16": id16,
    } for c in range(NCORES)]


def _finish(out, lengths):
    """out: [8, 96, 40] fp32 device results; returns the 4 model outputs."""
    vmin = np.empty((NCORES, MSL, M), np.float32)
    kmin = np.empty((NCORES, MSL, M), np.float32)
    for e in range(MSL):
        for mc in range(2):
            t = 2 * e + mc
            vmin[:, e, mc * MH:(mc + 1) * MH] = out[:, :, t]
            kmin[:, e, mc * MH:(mc + 1) * MH] = out[:, :, 20 + t]

    cstar = vmin.argmin(axis=0)                                # [10, 192]
    bv = np.take_along_axis(vmin, cstar[None], axis=0)[0]
    km = np.take_along_axis(kmin, cstar[None], axis=0)[0]
    lens = np.rint((km - bv) / np.float32(TAU)).astype(np.float32)
    lens = np.clip(lens, 0.0, float(MTL))

    # [e, m] -> [b, s, e]
    bv_bse = bv.reshape(MSL, BS, L).transpose(1, 2, 0)
    lens_bse = lens.reshape(MSL, BS, L).transpose(1, 2, 0)

    viable = (np.arange(L)[:, None] + np.arange(MSL)[None, :])[None] \
        < lengths[:, None, None]
    bvv = np.where(viable, bv_bse, np.float32(BIG))
    matched = bvv < np.float32(MATCH_THRESH)
    score = lens_bse * matched.astype(np.float32) * (np.float32(1.0) - bvv)

    sf = score.reshape(BS, -1)
    best_scores = sf.max(axis=-1).astype(np.float32)
    best_inds = sf.argmax(axis=-1).astype(np.int32)
    best_starts = best_inds // MSL
    best_ends = best_inds % MSL + best_starts
    matched_any = matched.reshape(BS, -1).any(axis=-1)
    return (best_scores, best_starts.astype(np.int32),
            best_ends.astype(np.int32), matched_any)


def kernel(word_repr, vocab_repr, lengths, vocab_length):
    lengths = np.asarray(lengths).astype(np.int64)
    in_maps = _host_prep(word_repr, vocab_repr, vocab_length)
    run, _nc = _get_runner()
    import time as _t
    t0 = _t.perf_counter()
    res = run(in_maps)
    _cache["last_device_s"] = _t.perf_counter() - t0
    _cache["last_in_maps"] = in_maps
    out = np.stack([res[c]["outT"] for c in range(NCORES)])
    return _finish(out, lengths)
